# revision 33
# baseline (speedup 1.0000x reference)
"""Trainium2 Bass kernel for nn_Attention_33638183862624 (linear/Taylor-softmax
attention). Data-parallel over batch: 16 batches -> 8 NeuronCores, 2 each.

Math per batch (C=512, N=4096, CQK=64), x flattened to [C, N]:
  Q = Wq x + bq; K = Wk x + bk           (Q,K: [64, N])
  Qn = Q / ||Q||_col; Kn = K / ||K||_col
  ksum = sum_n Kn[:, n]                  [64]
  denom = N + Qn^T ksum; tailor = 1/denom
  V = Wv x + bv                          (never materialized; fused:)
  matrix = Kn V^T = (Kn x^T) Wv^T + ksum bv^T            [64, 512]
  vsum   = V 1_N  = Wv (x 1_N) + N bv                    [512]
  out[c,n] = gamma * tailor[n] * (vsum[c] + sum_m matrix[m,c] Qn[m,n])

Implementation notes:
  * x is shipped to the device TWICE in fp16 (natural [C,N] and
    host-pre-transposed [N,C]) -- same DMA bytes as one fp32 copy, but the
    n-contraction product P = Kn_ext @ x^T needs no on-device transposes of x.
  * QK / P / mat matmuls run in fp16 (PSUM accumulation is fp32); the rest in
    float32r. Output is written fp16 and upcast on the host.
  * K is normalized in transposed layout; the per-partition 1/||K|| scalars are
    folded into the PSUM->SBUF cast-copies of K^T.
  * ksum rides in one [1,260] matmul per chunk (ones lhsT against the 4
    stacked K^T blocks); the ones-column entries accumulate to exactly N,
    which is precisely the row-64 coefficient the bv bias path needs.
  * One shared PSUM accumulator bank holds P (rows 0-64), ||Q||^2 rows
    (65-72), s8 = ksum^T Q rows (73-80) and the ksum row (81).
  * u := 1/(N*nq + s) and tailor = nq*u; rhs of the output matmul is
    [Q_raw; 1] * broadcast(u) so one reciprocal covers Q-normalize + tailor.
"""

import numpy as np

B, C, H, W = 16, 512, 64, 64
N = H * W          # 4096
CQK = C // 8       # 64
NCORES = 8
BLOC = B // NCORES  # 2 batches per core
NB = N // 512       # 8 n-chunks of 512
KC = C // 128       # 4 channel chunks of 128


# ---------------------------------------------------------------------------
# Walrus workaround: this container's walrus rejects >1 sync wait per
# instruction ("Too many sync wait commands"). (1) patch the TileContext tail
# drain to carry its waits on single-wait NOPs; (2) post-pass that rewrites
# any instruction with k>1 waits into k-1 single-wait NOPs + the instruction.
# ---------------------------------------------------------------------------

def _apply_tile_patches():
    import concourse.tile as tile
    from concourse import mybir
    from concourse.vector_clock import ScopedClock

    if getattr(tile.TileContext, "_drain_patched", False):
        return

    def _patched_drain_and_barrier(self, tick_clock, wait_clock):
        nop = self.nc.sync.nop(nofuse=True, hint="tail_drain_waits")
        wait_clock.add_sem_waits(
            nop.ins, ScopedClock({None: tick_clock.global_clock})
        )
        si = nop.ins.sync_info
        if si is not None and len(si.on_wait) > 1:
            waits = list(si.on_wait)
            nop.ins.sync_info = mybir.SyncInfo(on_wait=waits[:1], on_update=[])
            rest = waits[1:]
            while rest:
                n2 = self.nc.sync.nop(nofuse=True, hint="tail_drain_waits")
                n2.ins.sync_info = mybir.SyncInfo(on_wait=rest[:1], on_update=[])
                rest = rest[1:]
        self.nc.sync.drain()
        self.nc.all_engine_barrier()
        assert self.sems is not None
        popped = self.nc._tile_sem_poison_stack.pop()
        assert popped is self._sem_poison
        self.nc.clear_and_free_semaphores(list(self.sems.allocated().values()))
        self.nc.all_engine_barrier()

    tile.TileContext._drain_and_barrier = _patched_drain_and_barrier
    tile.TileContext._drain_patched = True


def _split_multi_waits(nc):
    from concourse import mybir

    counter = [0]
    for f in nc.m.functions:
        for bb in f.blocks:
            insts = bb.instructions
            if not any(
                i.sync_info is not None and len(i.sync_info.on_wait) > 1
                for i in insts
            ):
                continue
            new = []
            for ins in insts:
                si = ins.sync_info
                if si is not None and len(si.on_wait) > 1:
                    waits = list(si.on_wait)
                    for w in waits[:-1]:
                        counter[0] += 1
                        nop = mybir.InstNoOp(
                            name=f"I-wsplit-{counter[0]}", ins=[], outs=[]
                        )
                        nop.engine = ins.engine
                        nop.sync_info = mybir.SyncInfo(on_wait=[w], on_update=[])
                        new.append(nop)
                    ins.sync_info = mybir.SyncInfo(
                        on_wait=[waits[-1]], on_update=list(si.on_update)
                    )
                new.append(ins)
            bb.instructions = new


# ---------------------------------------------------------------------------
# Kernel body
# ---------------------------------------------------------------------------

def _build_module(use_bqk=True, use_bv=True):
    import concourse.bass as bass
    import concourse.tile as tile
    from concourse import mybir

    _apply_tile_patches()
    f32 = mybir.dt.float32
    f32r = mybir.dt.float32r
    f16 = mybir.dt.float16
    alu = mybir.AluOpType
    r = lambda ap: ap.bitcast(f32r)

    nc = bass.Bass("TRN2", target_bir_lowering=False, debug=False)

    # x/xt/out are tiled on the host so every data DMA is 128 fully
    # contiguous 4KB descriptors: [batch, chunk, partition, 2048 elems]
    x_d = nc.dram_tensor("x16", [BLOC, NB, 128, KC, 512], f16,
                         kind="ExternalInput").ap()
    xt_d = nc.dram_tensor("xt16", [BLOC, NB, 128, 4, 512], f16,
                          kind="ExternalInput").ap()
    wvt_d = nc.dram_tensor("wvt", [128, KC, 512], f16, kind="ExternalInput").ap()
    # all small constants packed into one fp16 + one fp32 DMA
    cf16_d = nc.dram_tensor("cf16", [128, 1729], f16, kind="ExternalInput").ap()
    cf32_d = nc.dram_tensor("cf32", [128, 196], f32, kind="ExternalInput").ap()
    ut_d = nc.dram_tensor("ut_scratch", [BLOC, NB, 512], f32,
                          kind="Internal").ap()
    out_d = nc.dram_tensor("out", [BLOC, NB, 128, KC, 512], f16,
                           kind="ExternalOutput").ap()

    from contextlib import ExitStack

    # PSUM matmul outputs must start at partition 0 (codegen rejects other
    # bases here): n2q8 and s8 time-share one [8, 512] bank region, ksum is
    # accumulated at A2 time from the retained knt tiles.

    with tile.TileContext(nc) as tc, ExitStack() as ctx, \
            nc.allow_low_precision(reason="fp16 operands validated vs reference"):
        consts = ctx.enter_context(tc.tile_pool(name="consts", bufs=1))
        xpool = ctx.enter_context(tc.tile_pool(name="xpool", bufs=3))
        xtpool = ctx.enter_context(tc.tile_pool(name="xtpool", bufs=3))
        batchp = ctx.enter_context(tc.tile_pool(name="batchp", bufs=2))
        work = ctx.enter_context(tc.tile_pool(name="work", bufs=3))
        outp = ctx.enter_context(tc.tile_pool(name="outp", bufs=3))
        pp_big = ctx.enter_context(tc.tile_pool(name="pp_big", bufs=5, space="PSUM"))
        pp_acc = ctx.enter_context(tc.tile_pool(name="pp_acc", bufs=1, space="PSUM"))
        pp_small = ctx.enter_context(
            tc.tile_pool(name="pp_small", bufs=1, space="PSUM")
        )

        # ---- constants: two packed DMAs; views into the packs ----
        cf16 = consts.tile([128, 1729], f16)
        nc.sync.dma_start(out=cf16, in_=cf16_d)
        wqkt = cf16[:, 0:512].rearrange("p (k c) -> p k c", c=128)
        ones128 = cf16[:, 512:513]
        bqk = cf16[0:1, 513:641]
        bv = cf16[0:1, 641:1153]
        onesn = cf16[0:1, 1153:1665]
        ident16 = cf16[0:64, 1665:1729]
        cf32 = consts.tile([128, 196], f32)
        nc.sync.dma_start(out=r(cf32), in_=r(cf32_d))
        ident = cf32[:, 0:128]
        sel8 = cf32[0:64, 128:192]
        onesq = cf32[0:64, 192:193]
        one1 = cf32[0:1, 193:195]
        gam128 = cf32[:, 195:196]
        wvt = consts.tile([128, KC, 512], f16)

        def alloc_state(b):
            st = {}
            st["q_raw"] = batchp.tile([65, N], f32, tag="q_raw",
                                      name=f"q_raw{b}")
            st["acc"] = pp_acc.tile([65, 512], f32, tag="acc",
                                    name=f"acc{b}")
            st["acc2"] = pp_acc.tile([8, 512], f32, tag="acc2",
                                     name=f"acc2{b}")
            st["ksum_row"] = batchp.tile([1, 65], f32, tag="ksum_row",
                                         name=f"ksum_row{b}")
            st["ksum_full"] = batchp.tile([65, 1], f32, tag="ksum_full",
                                          name=f"ksum_full{b}")
            st["ks8"] = batchp.tile([64, 64], f32, tag="ks8", name=f"ks8_{b}")
            nc.gpsimd.memset(st["ks8"], 0.0)
            st["mat_sb"] = batchp.tile([65, 512], f32, tag="mat_sb",
                                       name=f"mat_sb{b}")
            st["ut"] = batchp.tile([8, 512], f32, tag="ut", name=f"ut{b}")
            st["t2u"] = batchp.tile([65, NB, 512], f32, tag="t2u",
                                    name=f"t2u{b}")
            st["xh"] = {}
            st["xt"] = {}
            st["knt"] = {}
            st["qns"] = {}
            return st

        def emit_A_chunk(b, st, nb):
            sl = slice(512 * nb, 512 * (nb + 1))
            # x natural chunk [128, KC, 512] fp16 (one contiguous DMA)
            xh = xpool.tile([128, KC, 512], f16, tag="xh", name=f"xh{b}_{nb}")
            nc.sync.dma_start(out=xh, in_=x_d[b, nb])
            st["xh"][nb] = xh
            # x transposed slab [128, 4, 512] fp16 (one contiguous DMA)
            xt = xtpool.tile([128, 4, 512], f16, tag="xt", name=f"xt{b}_{nb}")
            nc.sync.dma_start(out=xt, in_=xt_d[b, nb])
            st["xt"][nb] = xt

            # QK = Wqk x (+ bqk) -> psum [128, 512] (rows 0-63 Q, 64-127 K)
            qk_ps = pp_big.tile([128, 512], f32, tag="big", name=f"qk{b}_{nb}")
            for k in range(KC):
                nc.tensor.matmul(
                    qk_ps, wqkt[:, k, :], xh[:, k, :],
                    start=(k == 0), stop=(k == KC - 1 and not use_bqk),
                )
            if use_bqk:
                nc.tensor.matmul(qk_ps, bqk, onesn, start=False, stop=True)

            # stash raw Q (DVE); K to sbuf for transposing (Pool); Q^2 (ACT)
            nc.vector.tensor_copy(
                out=r(st["q_raw"][0:64, sl]), in_=qk_ps[0:64, :],
            )
            k_sb = work.tile([64, 512], f16, tag="k_sb", bufs=3,
                             name=f"k_sb{b}_{nb}")
            nc.scalar.copy(out=k_sb, in_=qk_ps[64:128, :])
            sq_sb = work.tile([64, 512], f32, tag="sq", bufs=3,
                              name=f"sq{b}_{nb}")
            nc.scalar.square(out=r(sq_sb), in_=qk_ps[0:64, :])
            # ||Q_col||^2 for this chunk -> row nb of acc2
            nc.tensor.matmul(
                st["acc2"], r(sel8[:, 8 * nb:8 * (nb + 1)]), r(sq_sb),
                start=(nb == 0), stop=(nb == NB - 1), skip_group_check=True,
            )

            # K^T chunks (raw) via PE transpose
            kt_ps = pp_small.tile([128, 256], f16, tag="small",
                                  name=f"kt{b}_{nb}")
            for j in range(4):
                nc.tensor.transpose(
                    kt_ps[:, 64 * j:64 * (j + 1)],
                    k_sb[:, 128 * j:128 * (j + 1)],
                    ident16,
                )
            # column norms: square [128,256], reduce per 64-block, sqrt, recip
            ksq = work.tile([128, 4, 64], f32, tag="ksq", name=f"ksq{b}_{nb}")
            nc.scalar.square(out=ksq.rearrange("p j m -> p (j m)"), in_=kt_ps)
            nk2 = work.tile([128, 4], f32, tag="nk2", name=f"nk2{b}_{nb}")
            nc.vector.reduce_sum(out=nk2, in_=ksq, axis=mybir.AxisListType.X)
            nkt = work.tile([128, 4], f32, tag="nkt", name=f"nkt{b}_{nb}")
            nc.scalar.sqrt(out=nkt, in_=nk2)
            rkt = work.tile([128, 4], f32, tag="rkt", name=f"rkt{b}_{nb}")
            nc.vector.reciprocal(out=rkt, in_=nkt)
            # normalized K^T in fp16; col 64 of each block = 1 (vsum ride-along)
            knt = work.tile([128, 4, 65], f16, tag="knt", bufs=9,
                            name=f"knt{b}_{nb}")
            nc.gpsimd.memset(knt[:, :, 64:65], 1.0)
            for j in range(4):
                if j % 2 == 0:
                    nc.vector.tensor_scalar_mul(
                        out=knt[:, j, 0:64],
                        in0=kt_ps[:, 64 * j:64 * (j + 1)],
                        scalar1=rkt[:, j:j + 1],
                    )
                else:
                    nc.scalar.activation(
                        out=knt[:, j, 0:64],
                        in_=kt_ps[:, 64 * j:64 * (j + 1)],
                        func=mybir.ActivationFunctionType.Copy,
                        scale=rkt[:, j:j + 1],
                    )
            st["knt"][nb] = knt

        def emit_P_chunk(b, st, nb):
            # deferred one chunk so the knt producer chain has slack
            knt = st["knt"][nb]
            xt = st["xt"].pop(nb)
            for j in range(4):
                nc.tensor.matmul(
                    st["acc"], knt[:, j, :], xt[:, j, :],
                    start=(nb == 0 and j == 0),
                    stop=(nb == NB - 1 and j == 3),
                    skip_group_check=True,
                )

        def emit_A2(b, st):
            acc, acc2 = st["acc"], st["acc2"]
            # read ||Q||^2 rows out of acc2 before s8 reuses the region
            nq8 = work.tile([8, 512], f32, tag="nq8", name=f"nq8_{b}")
            nc.scalar.sqrt(out=nq8, in_=acc2)

            # ksum = sum_n Kn^T: ones^T against the 8 retained knt tiles
            ks260_ps = pp_small.tile([1, 260], f32, tag="small",
                                     name=f"ks260{b}")
            for nb in range(NB):
                knt = st["knt"].pop(nb)
                nc.tensor.matmul(
                    ks260_ps, ones128, knt.rearrange("p j m -> p (j m)"),
                    start=(nb == 0), stop=(nb == NB - 1),
                    skip_group_check=True,
                )
            # combine the 4 ksum j-blocks (col 64 of each block sums to N)
            ksr = work.tile([1, 260], f32, tag="ksr", name=f"ksr{b}")
            nc.vector.tensor_copy(out=r(ksr), in_=ks260_ps)
            ka = work.tile([1, 65], f32, tag="ka", name=f"ka{b}")
            nc.vector.tensor_add(
                out=ka, in0=ksr[:, 0:65], in1=ksr[:, 65:130]
            )
            kb = work.tile([1, 65], f32, tag="kb", name=f"kb{b}")
            nc.vector.tensor_add(
                out=kb, in0=ksr[:, 130:195], in1=ksr[:, 195:260],
            )
            nc.vector.tensor_add(out=r(st["ksum_row"]), in0=ka, in1=kb)
            # ksum as a column via a tiny matmul
            ksc_ps = pp_small.tile([65, 2], f32, tag="small", name=f"ksc{b}")
            nc.tensor.matmul(
                ksc_ps, r(st["ksum_row"]), r(one1), start=True, stop=True,
            )
            nc.vector.tensor_copy(out=r(st["ksum_full"]), in_=ksc_ps[:, 0:1])
            # ks8: column nb holds ksum in slot nb of each 8-block
            ks8 = st["ks8"]
            for nb in range(NB):
                nc.vector.tensor_copy(
                    out=r(ks8[:, 8 * nb + nb:8 * nb + nb + 1]),
                    in_=st["ksum_full"][0:64, :],
                )
            # s8[i, :] = ksum^T Q_raw(chunk i); reuses acc2 after the sqrt
            for nb in range(NB):
                sl = slice(512 * nb, 512 * (nb + 1))
                nc.tensor.matmul(
                    acc2,
                    r(ks8[:, 8 * nb:8 * (nb + 1)]), r(st["q_raw"][0:64, sl]),
                    start=(nb == 0), stop=(nb == NB - 1),
                    skip_group_check=True,
                )

            # mat = P Wv^T (+ ksum_ext bv^T), gamma folded in on the way out
            p_sb = work.tile([65, 512], f32, tag="p_sb", name=f"p_sb{b}")
            nc.vector.tensor_copy(out=r(p_sb), in_=acc)
            pt_ps = pp_small.tile([128, 264], f32, tag="small", name=f"pt{b}")
            for k in range(KC):
                nc.tensor.transpose(
                    r(pt_ps[:, 66 * k:66 * (k + 1)]),
                    r(p_sb[:, 128 * k:128 * (k + 1)]),
                    r(ident[0:65, 0:66]),
                )
            pt_sb = work.tile([128, KC, 65], f16, tag="pt_sb", name=f"ptsb{b}")
            nc.scalar.copy(
                out=pt_sb,
                in_=pt_ps[:].rearrange("p (k c) -> p k c", c=66)[:, :, 0:65],
            )
            mat_ps = pp_small.tile([65, 512], f32, tag="small",
                                   name=f"mat_ps{b}")
            for k in range(KC):
                nc.tensor.matmul(
                    mat_ps, pt_sb[:, k, :], wvt[:, k, :],
                    start=(k == 0), stop=(k == KC - 1 and not use_bv),
                    skip_group_check=True,
                )
            if use_bv:
                ksr16 = work.tile([1, 65], f16, tag="ksr16", name=f"ksr16{b}")
                nc.vector.tensor_copy(out=ksr16, in_=st["ksum_row"])
                nc.tensor.matmul(
                    mat_ps, ksr16, bv, start=False, stop=True,
                    skip_group_check=True,
                )
            nc.vector.tensor_scalar_mul(
                out=r(st["mat_sb"]), in0=mat_ps, scalar1=gam128[0:65, :]
            )

            # u = 1/(N*nq + s); q_raw row 64 := nq so qns row 64 = nq*u
            nc.scalar.dma_start(
                out=st["q_raw"][64:65, :].rearrange("o (c n) -> o c n", n=512),
                in_=nq8,
            )
            t1 = work.tile([8, 512], f32, tag="t1", name=f"t1_{b}")
            nc.vector.scalar_tensor_tensor(
                out=t1, in0=nq8, scalar=float(N), in1=acc2,
                op0=alu.mult, op1=alu.add,
            )
            nc.vector.reciprocal(out=r(st["ut"]), in_=t1)
            # broadcast u to 65 partitions via a DRAM bounce (stride-0 read)
            nc.scalar.dma_start(out=ut_d[b], in_=st["ut"])
            nc.scalar.dma_start(
                out=st["t2u"],
                in_=bass.AP(
                    tensor=ut_d.tensor, offset=ut_d[b, 0, 0].offset,
                    ap=[[0, 65], [1, N]],
                ),
            )

        def emit_B_t2(b, st, nb):
            sl = slice(512 * nb, 512 * (nb + 1))
            qns = work.tile([65, 512], f32, tag="qns", bufs=3,
                            name=f"qns{b}_{nb}")
            nc.vector.tensor_mul(out=r(qns), in0=st["q_raw"][:, sl],
                                 in1=st["t2u"][:, nb, :])
            st["qns"][nb] = qns

        def emit_B_out(b, st, nb):
            sl = slice(512 * nb, 512 * (nb + 1))
            qns = st["qns"].pop(nb)
            o16 = outp.tile([128, KC, 512], f16, tag="o", name=f"o16_{b}_{nb}")
            for cb in range(KC):
                o_ps = pp_big.tile([128, 512], f32, tag="big",
                                   name=f"o_ps{b}_{nb}_{cb}")
                nc.tensor.matmul(
                    o_ps, r(st["mat_sb"][:, 128 * cb:128 * (cb + 1)]), r(qns),
                    start=True, stop=True,
                )
                if cb % 2 == 0:
                    nc.vector.tensor_copy(out=o16[:, cb, :], in_=o_ps)
                else:
                    nc.scalar.copy(out=o16[:, cb, :], in_=o_ps)
            nc.scalar.dma_start(out=out_d[b, nb], in_=o16)

        # Software pipeline: batch b stage A interleaves with batch b-1
        # stage B; P and out matmuls trail their producers by one chunk so
        # the in-order PE stream never waits on DVE/ACT chains.
        states = {}
        for b in range(BLOC):
            states[b] = alloc_state(b)
            for nb in range(NB):
                emit_A_chunk(b, states[b], nb)
                if b == 0 and nb == 3:
                    # wvt is first needed at A2(0); slot its load mid-stream
                    nc.sync.dma_start(out=wvt, in_=wvt_d)
                if nb > 0:
                    emit_P_chunk(b, states[b], nb - 1)
                if b > 0:
                    emit_B_t2(b - 1, states[b - 1], nb)
                    if nb > 0:
                        emit_B_out(b - 1, states[b - 1], nb - 1)
            emit_P_chunk(b, states[b], NB - 1)
            if b > 0:
                emit_B_out(b - 1, states[b - 1], NB - 1)
            emit_A2(b, states[b])
        stl = states[BLOC - 1]
        for nb in range(NB):
            emit_B_t2(BLOC - 1, stl, nb)
            if nb > 0:
                emit_B_out(BLOC - 1, stl, nb - 1)
        emit_B_out(BLOC - 1, stl, NB - 1)

    _split_multi_waits(nc)
    return nc


_CACHE = {}


def _get_module(use_bqk, use_bv):
    key = (use_bqk, use_bv)
    if key not in _CACHE:
        _CACHE[key] = _build_module(*key)
    return _CACHE[key]


def _host_inputs(x, Wq, bq, Wk, bk, Wv, bv, gamma):
    x = np.ascontiguousarray(np.asarray(x, dtype=np.float32)).reshape(B, C, N)
    Wq = np.asarray(Wq, dtype=np.float32)
    Wk = np.asarray(Wk, dtype=np.float32)
    Wv = np.asarray(Wv, dtype=np.float32)
    bq = np.asarray(bq, dtype=np.float32)
    bk = np.asarray(bk, dtype=np.float32)
    bvv = np.asarray(bv, dtype=np.float32)
    gamma = np.asarray(gamma, dtype=np.float32)

    x16 = x.astype(np.float16)
    # tiled layouts: one contiguous 4KB-per-partition DMA per chunk
    # x16_t[b, nb, p, k, c] = x16[b, 128k+p, 512nb+c]
    x16_t = np.ascontiguousarray(
        x16.reshape(B, KC, 128, NB, 512).transpose(0, 3, 2, 1, 4)
    )                                                     # [B, NB, 128, KC, 512]
    # xt16_t[b, nb, p, j, c] = x16[b, c, 512nb+128j+p]
    xt16_t = np.ascontiguousarray(
        x16.transpose(0, 2, 1).reshape(B, NB, 4, 128, C).transpose(0, 1, 3, 2, 4)
    )                                                     # [B, NB, 128, 4, C]

    wqk = np.concatenate([Wq, Wk], axis=0)                # [128, 512]
    wqkt = np.ascontiguousarray(
        wqk.T.reshape(KC, 128, 128).transpose(1, 0, 2)
    ).astype(np.float16)                                  # [128, KC, 128]
    wvt = np.ascontiguousarray(
        Wv.T.reshape(KC, 128, 512).transpose(1, 0, 2)
    ).astype(np.float16)                                  # [128, KC, 512]
    cf16 = np.zeros((128, 1729), np.float16)
    cf16[:, 0:512] = wqkt.reshape(128, 512)
    cf16[:, 512] = 1.0                                    # ones128
    cf16[0, 513:641] = np.concatenate([bq, bk]).astype(np.float16)
    cf16[0, 641:1153] = bvv.astype(np.float16)
    cf16[0, 1153:1665] = 1.0                              # onesn
    cf16[0:64, 1665:1729] = np.eye(64, dtype=np.float16)  # ident16
    cf32 = np.zeros((128, 196), np.float32)
    cf32[:, 0:128] = np.eye(128, dtype=np.float32)        # ident
    for nb in range(8):
        cf32[0:64, 128 + 8 * nb + nb] = 1.0               # sel8
    cf32[0:64, 192] = 1.0                                 # onesq
    cf32[0, 193:195] = 1.0                                # one1
    cf32[:, 195] = gamma.reshape(-1)[0]                   # gam128

    shared = dict(wvt=wvt, cf16=cf16, cf32=cf32)
    in_maps = []
    for c in range(NCORES):
        m = dict(shared)
        m["x16"] = np.ascontiguousarray(x16_t[c * BLOC:(c + 1) * BLOC])
        m["xt16"] = np.ascontiguousarray(xt16_t[c * BLOC:(c + 1) * BLOC])
        in_maps.append(m)
    return in_maps


def run_on_device(in_maps, **kw):
    from concourse.bass_utils import run_bass_kernel_spmd

    m = in_maps[0]
    use_bqk = bool(np.any(m["cf16"][0, 513:641]))
    use_bv = bool(np.any(m["cf16"][0, 641:1153]))
    nc = _get_module(use_bqk, use_bv)
    return run_bass_kernel_spmd(nc, in_maps, core_ids=list(range(NCORES)), **kw)


def _unpack_out(res):
    out = np.concatenate([r["out"] for r in res.results], axis=0)
    # out[b, nb, p, k, c] -> [b, 128k+p, 512nb+c]
    out = out.reshape(B, NB, 128, KC, 512).transpose(0, 3, 2, 1, 4)
    return np.ascontiguousarray(out).reshape(B, C, H, W).astype(np.float32)


def kernel(x, Wq, bq, Wk, bk, Wv, bv, gamma):
    in_maps = _host_inputs(x, Wq, bq, Wk, bk, Wv, bv, gamma)
    res = run_on_device(in_maps)
    return _unpack_out(res)


# revision 34
# speedup vs baseline: 1.1674x; 1.1674x over previous
"""Trainium2 Bass kernel for nn_Attention_33638183862624 (linear/Taylor-softmax
attention). Data-parallel over batch: 16 batches -> 8 NeuronCores, 2 each.

Math per batch (C=512, N=4096, CQK=64), x flattened to [C, N]:
  Q = Wq x + bq; K = Wk x + bk           (Q,K: [64, N])
  Qn = Q / ||Q||_col; Kn = K / ||K||_col
  ksum = sum_n Kn[:, n]                  [64]
  denom = N + Qn^T ksum; tailor = 1/denom
  V = Wv x + bv                          (never materialized; fused:)
  matrix = Kn V^T = (Kn x^T) Wv^T + ksum bv^T            [64, 512]
  vsum   = V 1_N  = Wv (x 1_N) + N bv                    [512]
  out[c,n] = gamma * tailor[n] * (vsum[c] + sum_m matrix[m,c] Qn[m,n])

Implementation notes:
  * x is shipped to the device TWICE in fp16 (natural [C,N] and
    host-pre-transposed [N,C]) -- same DMA bytes as one fp32 copy, but the
    n-contraction product P = Kn_ext @ x^T needs no on-device transposes of x.
  * QK / P / mat matmuls run in fp16 (PSUM accumulation is fp32); the rest in
    float32r. Output is written fp16 and upcast on the host.
  * K is normalized in transposed layout; the per-partition 1/||K|| scalars are
    folded into the PSUM->SBUF cast-copies of K^T.
  * ksum rides in one [1,260] matmul per chunk (ones lhsT against the 4
    stacked K^T blocks); the ones-column entries accumulate to exactly N,
    which is precisely the row-64 coefficient the bv bias path needs.
  * One shared PSUM accumulator bank holds P (rows 0-64), ||Q||^2 rows
    (65-72), s8 = ksum^T Q rows (73-80) and the ksum row (81).
  * u := 1/(N*nq + s) and tailor = nq*u; rhs of the output matmul is
    [Q_raw; 1] * broadcast(u) so one reciprocal covers Q-normalize + tailor.
"""

import numpy as np

B, C, H, W = 16, 512, 64, 64
N = H * W          # 4096
CQK = C // 8       # 64
NCORES = 8
BLOC = B // NCORES  # 2 batches per core
NB = N // 512       # 8 n-chunks of 512
KC = C // 128       # 4 channel chunks of 128


# ---------------------------------------------------------------------------
# Walrus workaround: this container's walrus rejects >1 sync wait per
# instruction ("Too many sync wait commands"). (1) patch the TileContext tail
# drain to carry its waits on single-wait NOPs; (2) post-pass that rewrites
# any instruction with k>1 waits into k-1 single-wait NOPs + the instruction.
# ---------------------------------------------------------------------------

def _apply_tile_patches():
    import concourse.tile as tile
    from concourse import mybir
    from concourse.vector_clock import ScopedClock

    if getattr(tile.TileContext, "_drain_patched", False):
        return

    def _patched_drain_and_barrier(self, tick_clock, wait_clock):
        nop = self.nc.sync.nop(nofuse=True, hint="tail_drain_waits")
        wait_clock.add_sem_waits(
            nop.ins, ScopedClock({None: tick_clock.global_clock})
        )
        si = nop.ins.sync_info
        if si is not None and len(si.on_wait) > 1:
            waits = list(si.on_wait)
            nop.ins.sync_info = mybir.SyncInfo(on_wait=waits[:1], on_update=[])
            rest = waits[1:]
            while rest:
                n2 = self.nc.sync.nop(nofuse=True, hint="tail_drain_waits")
                n2.ins.sync_info = mybir.SyncInfo(on_wait=rest[:1], on_update=[])
                rest = rest[1:]
        self.nc.sync.drain()
        self.nc.all_engine_barrier()
        assert self.sems is not None
        popped = self.nc._tile_sem_poison_stack.pop()
        assert popped is self._sem_poison
        self.nc.clear_and_free_semaphores(list(self.sems.allocated().values()))
        self.nc.all_engine_barrier()

    tile.TileContext._drain_and_barrier = _patched_drain_and_barrier
    tile.TileContext._drain_patched = True


def _split_multi_waits(nc):
    from concourse import mybir

    counter = [0]
    for f in nc.m.functions:
        for bb in f.blocks:
            insts = bb.instructions
            if not any(
                i.sync_info is not None and len(i.sync_info.on_wait) > 1
                for i in insts
            ):
                continue
            new = []
            for ins in insts:
                si = ins.sync_info
                if si is not None and len(si.on_wait) > 1:
                    waits = list(si.on_wait)
                    for w in waits[:-1]:
                        counter[0] += 1
                        nop = mybir.InstNoOp(
                            name=f"I-wsplit-{counter[0]}", ins=[], outs=[]
                        )
                        nop.engine = ins.engine
                        nop.sync_info = mybir.SyncInfo(on_wait=[w], on_update=[])
                        new.append(nop)
                    ins.sync_info = mybir.SyncInfo(
                        on_wait=[waits[-1]], on_update=list(si.on_update)
                    )
                new.append(ins)
            bb.instructions = new


# ---------------------------------------------------------------------------
# Kernel body
# ---------------------------------------------------------------------------

def _build_module(use_bqk=True, use_bv=True):
    import concourse.bass as bass
    import concourse.tile as tile
    from concourse import mybir

    _apply_tile_patches()
    f32 = mybir.dt.float32
    f32r = mybir.dt.float32r
    f16 = mybir.dt.float16
    alu = mybir.AluOpType
    r = lambda ap: ap.bitcast(f32r)

    nc = bass.Bass("TRN2", target_bir_lowering=False, debug=False)

    # x/xt/out are tiled on the host so every data DMA is 128 fully
    # contiguous 4KB descriptors: [batch, chunk, partition, 2048 elems]
    x_d = nc.dram_tensor("x16", [BLOC, NB, 128, KC, 512], f16,
                         kind="ExternalInput").ap()
    xt_d = nc.dram_tensor("xt16", [BLOC, NB, 128, 4, 512], f16,
                          kind="ExternalInput").ap()
    wvt_d = nc.dram_tensor("wvt", [128, KC, 512], f16, kind="ExternalInput").ap()
    # all small constants packed into one fp16 + one fp32 DMA
    cf16_d = nc.dram_tensor("cf16", [128, 1729], f16, kind="ExternalInput").ap()
    cf32_d = nc.dram_tensor("cf32", [128, 716], f32, kind="ExternalInput").ap()
    out_d = nc.dram_tensor("out", [BLOC, NB, 128, KC, 512], f16,
                           kind="ExternalOutput").ap()

    from contextlib import ExitStack

    # PSUM matmul outputs must start at partition 0 (codegen rejects other
    # bases here): n2q8 and s8 time-share one [8, 512] bank region, ksum is
    # accumulated at A2 time from the retained knt tiles.

    with tile.TileContext(nc) as tc, ExitStack() as ctx, \
            nc.allow_low_precision(reason="fp16 operands validated vs reference"):
        consts = ctx.enter_context(tc.tile_pool(name="consts", bufs=1))
        xpool = ctx.enter_context(tc.tile_pool(name="xpool", bufs=3))
        xtpool = ctx.enter_context(tc.tile_pool(name="xtpool", bufs=3))
        batchp = ctx.enter_context(tc.tile_pool(name="batchp", bufs=2))
        work = ctx.enter_context(tc.tile_pool(name="work", bufs=3))
        outp = ctx.enter_context(tc.tile_pool(name="outp", bufs=3))
        pp_big = ctx.enter_context(tc.tile_pool(name="pp_big", bufs=5, space="PSUM"))
        pp_acc = ctx.enter_context(tc.tile_pool(name="pp_acc", bufs=1, space="PSUM"))
        pp_small = ctx.enter_context(
            tc.tile_pool(name="pp_small", bufs=1, space="PSUM")
        )

        # ---- constants: two packed DMAs; views into the packs ----
        cf16 = consts.tile([128, 1729], f16)
        nc.sync.dma_start(out=cf16, in_=cf16_d)
        wqkt = cf16[:, 0:512].rearrange("p (k c) -> p k c", c=128)
        ones128 = cf16[:, 512:513]
        bqk = cf16[0:1, 513:641]
        bv = cf16[0:1, 641:1153]
        onesn = cf16[0:1, 1153:1665]
        ident16 = cf16[0:64, 1665:1729]
        cf32 = consts.tile([128, 716], f32)
        nc.sync.dma_start(out=r(cf32), in_=r(cf32_d))
        ident = cf32[:, 0:128]
        sel8 = cf32[0:64, 128:192]
        onesq = cf32[0:64, 192:193]
        one1 = cf32[0:1, 193:195]
        gam128 = cf32[:, 195:196]
        selab = cf32[0:8, 196:716]
        wvt = consts.tile([128, KC, 512], f16)

        def alloc_state(b):
            st = {}
            st["q_raw"] = batchp.tile([65, N], f32, tag="q_raw",
                                      name=f"q_raw{b}")
            st["acc"] = pp_acc.tile([65, 512], f32, tag="acc",
                                    name=f"acc{b}")
            st["acc2"] = pp_acc.tile([8, 512], f32, tag="acc2",
                                     name=f"acc2{b}")
            st["ksum_row"] = batchp.tile([1, 65], f32, tag="ksum_row",
                                         name=f"ksum_row{b}")
            st["ksum_full"] = batchp.tile([65, 1], f32, tag="ksum_full",
                                          name=f"ksum_full{b}")
            st["ks8"] = batchp.tile([64, 64], f32, tag="ks8", name=f"ks8_{b}")
            nc.gpsimd.memset(st["ks8"], 0.0)
            st["mat_sb"] = batchp.tile([65, 512], f32, tag="mat_sb",
                                       name=f"mat_sb{b}")
            st["ut"] = batchp.tile([8, 512], f32, tag="ut", name=f"ut{b}")
            st["xh"] = {}
            st["xt"] = {}
            st["knt"] = {}
            st["qns"] = {}
            return st

        def emit_A_chunk(b, st, nb):
            sl = slice(512 * nb, 512 * (nb + 1))
            # x natural chunk [128, KC, 512] fp16 (one contiguous DMA)
            xh = xpool.tile([128, KC, 512], f16, tag="xh", name=f"xh{b}_{nb}")
            nc.sync.dma_start(out=xh, in_=x_d[b, nb])
            st["xh"][nb] = xh
            # x transposed slab [128, 4, 512] fp16 (one contiguous DMA)
            xt = xtpool.tile([128, 4, 512], f16, tag="xt", name=f"xt{b}_{nb}")
            nc.sync.dma_start(out=xt, in_=xt_d[b, nb])
            st["xt"][nb] = xt

            # QK = Wqk x (+ bqk) -> psum [128, 512] (rows 0-63 Q, 64-127 K)
            qk_ps = pp_big.tile([128, 512], f32, tag="big", name=f"qk{b}_{nb}")
            for k in range(KC):
                nc.tensor.matmul(
                    qk_ps, wqkt[:, k, :], xh[:, k, :],
                    start=(k == 0), stop=(k == KC - 1 and not use_bqk),
                )
            if use_bqk:
                nc.tensor.matmul(qk_ps, bqk, onesn, start=False, stop=True)

            # stash raw Q (DVE); K to sbuf for transposing (Pool); Q^2 (ACT)
            nc.vector.tensor_copy(
                out=r(st["q_raw"][0:64, sl]), in_=qk_ps[0:64, :],
            )
            k_sb = work.tile([64, 512], f16, tag="k_sb", bufs=3,
                             name=f"k_sb{b}_{nb}")
            nc.scalar.copy(out=k_sb, in_=qk_ps[64:128, :])
            sq_sb = work.tile([64, 512], f32, tag="sq", bufs=3,
                              name=f"sq{b}_{nb}")
            nc.scalar.square(out=r(sq_sb), in_=qk_ps[0:64, :])
            # ||Q_col||^2 for this chunk -> row nb of acc2
            nc.tensor.matmul(
                st["acc2"], r(sel8[:, 8 * nb:8 * (nb + 1)]), r(sq_sb),
                start=(nb == 0), stop=(nb == NB - 1), skip_group_check=True,
            )

            # K^T chunks (raw) via PE transpose
            kt_ps = pp_small.tile([128, 256], f16, tag="small",
                                  name=f"kt{b}_{nb}")
            for j in range(4):
                nc.tensor.transpose(
                    kt_ps[:, 64 * j:64 * (j + 1)],
                    k_sb[:, 128 * j:128 * (j + 1)],
                    ident16,
                )
            # column norms: square [128,256], reduce per 64-block, sqrt, recip
            ksq = work.tile([128, 4, 64], f32, tag="ksq", name=f"ksq{b}_{nb}")
            nc.scalar.square(out=ksq.rearrange("p j m -> p (j m)"), in_=kt_ps)
            nk2 = work.tile([128, 4], f32, tag="nk2", name=f"nk2{b}_{nb}")
            nc.vector.reduce_sum(out=nk2, in_=ksq, axis=mybir.AxisListType.X)
            nkt = work.tile([128, 4], f32, tag="nkt", name=f"nkt{b}_{nb}")
            nc.scalar.sqrt(out=nkt, in_=nk2)
            rkt = work.tile([128, 4], f32, tag="rkt", name=f"rkt{b}_{nb}")
            nc.vector.reciprocal(out=rkt, in_=nkt)
            # normalized K^T in fp16; col 64 of each block = 1 (vsum ride-along)
            knt = work.tile([128, 4, 65], f16, tag="knt", bufs=9,
                            name=f"knt{b}_{nb}")
            nc.gpsimd.memset(knt[:, :, 64:65], 1.0)
            for j in range(4):
                if j % 2 == 0:
                    nc.vector.tensor_scalar_mul(
                        out=knt[:, j, 0:64],
                        in0=kt_ps[:, 64 * j:64 * (j + 1)],
                        scalar1=rkt[:, j:j + 1],
                    )
                else:
                    nc.scalar.activation(
                        out=knt[:, j, 0:64],
                        in_=kt_ps[:, 64 * j:64 * (j + 1)],
                        func=mybir.ActivationFunctionType.Copy,
                        scale=rkt[:, j:j + 1],
                    )
            st["knt"][nb] = knt

        def emit_P_chunk(b, st, nb):
            # deferred one chunk so the knt producer chain has slack
            knt = st["knt"][nb]
            xt = st["xt"].pop(nb)
            for j in range(4):
                nc.tensor.matmul(
                    st["acc"], knt[:, j, :], xt[:, j, :],
                    start=(nb == 0 and j == 0),
                    stop=(nb == NB - 1 and j == 3),
                    skip_group_check=True,
                )

        def emit_A2(b, st):
            acc, acc2 = st["acc"], st["acc2"]
            # read ||Q||^2 rows out of acc2 before s8 reuses the region
            nq8 = work.tile([8, 512], f32, tag="nq8", name=f"nq8_{b}")
            nc.scalar.sqrt(out=nq8, in_=acc2)

            # ksum = sum_n Kn^T: ones^T against the 8 retained knt tiles
            ks260_ps = pp_small.tile([1, 260], f32, tag="small",
                                     name=f"ks260{b}")
            for nb in range(NB):
                knt = st["knt"].pop(nb)
                nc.tensor.matmul(
                    ks260_ps, ones128, knt.rearrange("p j m -> p (j m)"),
                    start=(nb == 0), stop=(nb == NB - 1),
                    skip_group_check=True,
                )
            # combine the 4 ksum j-blocks (col 64 of each block sums to N)
            ksr = work.tile([1, 260], f32, tag="ksr", name=f"ksr{b}")
            nc.vector.tensor_copy(out=r(ksr), in_=ks260_ps)
            ka = work.tile([1, 65], f32, tag="ka", name=f"ka{b}")
            nc.vector.tensor_add(
                out=ka, in0=ksr[:, 0:65], in1=ksr[:, 65:130]
            )
            kb = work.tile([1, 65], f32, tag="kb", name=f"kb{b}")
            nc.vector.tensor_add(
                out=kb, in0=ksr[:, 130:195], in1=ksr[:, 195:260],
            )
            nc.vector.tensor_add(out=r(st["ksum_row"]), in0=ka, in1=kb)
            # ksum as a column via a tiny matmul
            ksc_ps = pp_small.tile([65, 2], f32, tag="small", name=f"ksc{b}")
            nc.tensor.matmul(
                ksc_ps, r(st["ksum_row"]), r(one1), start=True, stop=True,
            )
            nc.vector.tensor_copy(out=r(st["ksum_full"]), in_=ksc_ps[:, 0:1])
            # ks8: column nb holds ksum in slot nb of each 8-block
            ks8 = st["ks8"]
            for nb in range(NB):
                nc.vector.tensor_copy(
                    out=r(ks8[:, 8 * nb + nb:8 * nb + nb + 1]),
                    in_=st["ksum_full"][0:64, :],
                )
            # s8[i, :] = ksum^T Q_raw(chunk i); reuses acc2 after the sqrt
            for nb in range(NB):
                sl = slice(512 * nb, 512 * (nb + 1))
                nc.tensor.matmul(
                    acc2,
                    r(ks8[:, 8 * nb:8 * (nb + 1)]), r(st["q_raw"][0:64, sl]),
                    start=(nb == 0), stop=(nb == NB - 1),
                    skip_group_check=True,
                )

            # mat = P Wv^T (+ ksum_ext bv^T), gamma folded in on the way out
            p_sb = work.tile([65, 512], f32, tag="p_sb", name=f"p_sb{b}")
            nc.vector.tensor_copy(out=r(p_sb), in_=acc)
            pt_ps = pp_small.tile([128, 264], f32, tag="small", name=f"pt{b}")
            for k in range(KC):
                nc.tensor.transpose(
                    r(pt_ps[:, 66 * k:66 * (k + 1)]),
                    r(p_sb[:, 128 * k:128 * (k + 1)]),
                    r(ident[0:65, 0:66]),
                )
            pt_sb = work.tile([128, KC, 65], f16, tag="pt_sb", name=f"ptsb{b}")
            nc.scalar.copy(
                out=pt_sb,
                in_=pt_ps[:].rearrange("p (k c) -> p k c", c=66)[:, :, 0:65],
            )
            mat_ps = pp_small.tile([65, 512], f32, tag="small",
                                   name=f"mat_ps{b}")
            for k in range(KC):
                nc.tensor.matmul(
                    mat_ps, pt_sb[:, k, :], wvt[:, k, :],
                    start=(k == 0), stop=(k == KC - 1 and not use_bv),
                    skip_group_check=True,
                )
            if use_bv:
                ksr16 = work.tile([1, 65], f16, tag="ksr16", name=f"ksr16{b}")
                nc.vector.tensor_copy(out=ksr16, in_=st["ksum_row"])
                nc.tensor.matmul(
                    mat_ps, ksr16, bv, start=False, stop=True,
                    skip_group_check=True,
                )
            nc.vector.tensor_scalar_mul(
                out=r(st["mat_sb"]), in0=mat_ps, scalar1=gam128[0:65, :]
            )

            # u = 1/(N*nq + s); q_raw row 64 := nq so qns row 64 = nq*u
            nc.scalar.dma_start(
                out=st["q_raw"][64:65, :].rearrange("o (c n) -> o c n", n=512),
                in_=nq8,
            )
            t1 = work.tile([8, 512], f32, tag="t1", name=f"t1_{b}")
            nc.vector.scalar_tensor_tensor(
                out=t1, in0=nq8, scalar=float(N), in1=acc2,
                op0=alu.mult, op1=alu.add,
            )
            nc.vector.reciprocal(out=r(st["ut"]), in_=t1)

        def emit_B_t2(b, st, nb):
            sl = slice(512 * nb, 512 * (nb + 1))
            # broadcast u(chunk nb) to all 65 partitions (row 64 of q_raw
            # holds nq, so the product's row 64 is tailor = nq*u)
            t2_ps = pp_big.tile([65, 512], f32, tag="big", name=f"t2_{b}_{nb}")
            nc.tensor.matmul(
                t2_ps, r(selab[:, 65 * nb:65 * (nb + 1)]), r(st["ut"]),
                start=True, stop=True,
            )
            qns = work.tile([65, 512], f32, tag="qns", bufs=3,
                            name=f"qns{b}_{nb}")
            nc.vector.tensor_mul(out=r(qns), in0=st["q_raw"][:, sl], in1=t2_ps)
            st["qns"][nb] = qns

        def emit_B_out(b, st, nb):
            sl = slice(512 * nb, 512 * (nb + 1))
            qns = st["qns"].pop(nb)
            o16 = outp.tile([128, KC, 512], f16, tag="o", name=f"o16_{b}_{nb}")
            for cb in range(KC):
                o_ps = pp_big.tile([128, 512], f32, tag="big",
                                   name=f"o_ps{b}_{nb}_{cb}")
                nc.tensor.matmul(
                    o_ps, r(st["mat_sb"][:, 128 * cb:128 * (cb + 1)]), r(qns),
                    start=True, stop=True,
                )
                if cb % 2 == 0:
                    nc.vector.tensor_copy(out=o16[:, cb, :], in_=o_ps)
                else:
                    nc.scalar.copy(out=o16[:, cb, :], in_=o_ps)
            nc.scalar.dma_start(out=out_d[b, nb], in_=o16)

        # Software pipeline: batch b stage A interleaves with batch b-1
        # stage B; P and out matmuls trail their producers by one chunk so
        # the in-order PE stream never waits on DVE/ACT chains.
        states = {}
        for b in range(BLOC):
            states[b] = alloc_state(b)
            for nb in range(NB):
                emit_A_chunk(b, states[b], nb)
                if b == 0 and nb == 3:
                    # wvt is first needed at A2(0); slot its load mid-stream
                    nc.sync.dma_start(out=wvt, in_=wvt_d)
                if nb > 0:
                    emit_P_chunk(b, states[b], nb - 1)
                if b > 0:
                    emit_B_t2(b - 1, states[b - 1], nb)
                    if nb > 0:
                        emit_B_out(b - 1, states[b - 1], nb - 1)
            emit_P_chunk(b, states[b], NB - 1)
            if b > 0:
                emit_B_out(b - 1, states[b - 1], NB - 1)
            emit_A2(b, states[b])
        stl = states[BLOC - 1]
        for nb in range(NB):
            emit_B_t2(BLOC - 1, stl, nb)
            if nb > 0:
                emit_B_out(BLOC - 1, stl, nb - 1)
        emit_B_out(BLOC - 1, stl, NB - 1)

    _split_multi_waits(nc)
    return nc


_CACHE = {}


def _get_module(use_bqk, use_bv):
    key = (use_bqk, use_bv)
    if key not in _CACHE:
        _CACHE[key] = _build_module(*key)
    return _CACHE[key]


def _host_inputs(x, Wq, bq, Wk, bk, Wv, bv, gamma):
    x = np.ascontiguousarray(np.asarray(x, dtype=np.float32)).reshape(B, C, N)
    Wq = np.asarray(Wq, dtype=np.float32)
    Wk = np.asarray(Wk, dtype=np.float32)
    Wv = np.asarray(Wv, dtype=np.float32)
    bq = np.asarray(bq, dtype=np.float32)
    bk = np.asarray(bk, dtype=np.float32)
    bvv = np.asarray(bv, dtype=np.float32)
    gamma = np.asarray(gamma, dtype=np.float32)

    x16 = x.astype(np.float16)
    # tiled layouts: one contiguous 4KB-per-partition DMA per chunk
    # x16_t[b, nb, p, k, c] = x16[b, 128k+p, 512nb+c]
    x16_t = np.ascontiguousarray(
        x16.reshape(B, KC, 128, NB, 512).transpose(0, 3, 2, 1, 4)
    )                                                     # [B, NB, 128, KC, 512]
    # xt16_t[b, nb, p, j, c] = x16[b, c, 512nb+128j+p]
    xt16_t = np.ascontiguousarray(
        x16.transpose(0, 2, 1).reshape(B, NB, 4, 128, C).transpose(0, 1, 3, 2, 4)
    )                                                     # [B, NB, 128, 4, C]

    wqk = np.concatenate([Wq, Wk], axis=0)                # [128, 512]
    wqkt = np.ascontiguousarray(
        wqk.T.reshape(KC, 128, 128).transpose(1, 0, 2)
    ).astype(np.float16)                                  # [128, KC, 128]
    wvt = np.ascontiguousarray(
        Wv.T.reshape(KC, 128, 512).transpose(1, 0, 2)
    ).astype(np.float16)                                  # [128, KC, 512]
    cf16 = np.zeros((128, 1729), np.float16)
    cf16[:, 0:512] = wqkt.reshape(128, 512)
    cf16[:, 512] = 1.0                                    # ones128
    cf16[0, 513:641] = np.concatenate([bq, bk]).astype(np.float16)
    cf16[0, 641:1153] = bvv.astype(np.float16)
    cf16[0, 1153:1665] = 1.0                              # onesn
    cf16[0:64, 1665:1729] = np.eye(64, dtype=np.float16)  # ident16
    cf32 = np.zeros((128, 716), np.float32)
    cf32[:, 0:128] = np.eye(128, dtype=np.float32)        # ident
    for nb in range(8):
        cf32[0:64, 128 + 8 * nb + nb] = 1.0               # sel8
        cf32[nb, 196 + 65 * nb:196 + 65 * nb + 65] = 1.0  # selab (u -> 65 rows)
    cf32[0:64, 192] = 1.0                                 # onesq
    cf32[0, 193:195] = 1.0                                # one1
    cf32[:, 195] = gamma.reshape(-1)[0]                   # gam128

    shared = dict(wvt=wvt, cf16=cf16, cf32=cf32)
    in_maps = []
    for c in range(NCORES):
        m = dict(shared)
        m["x16"] = np.ascontiguousarray(x16_t[c * BLOC:(c + 1) * BLOC])
        m["xt16"] = np.ascontiguousarray(xt16_t[c * BLOC:(c + 1) * BLOC])
        in_maps.append(m)
    return in_maps


def run_on_device(in_maps, **kw):
    from concourse.bass_utils import run_bass_kernel_spmd

    m = in_maps[0]
    use_bqk = bool(np.any(m["cf16"][0, 513:641]))
    use_bv = bool(np.any(m["cf16"][0, 641:1153]))
    nc = _get_module(use_bqk, use_bv)
    return run_bass_kernel_spmd(nc, in_maps, core_ids=list(range(NCORES)), **kw)


def _unpack_out(res):
    out = np.concatenate([r["out"] for r in res.results], axis=0)
    # out[b, nb, p, k, c] -> [b, 128k+p, 512nb+c]
    out = out.reshape(B, NB, 128, KC, 512).transpose(0, 3, 2, 1, 4)
    return np.ascontiguousarray(out).reshape(B, C, H, W).astype(np.float32)


def kernel(x, Wq, bq, Wk, bk, Wv, bv, gamma):
    in_maps = _host_inputs(x, Wq, bq, Wk, bk, Wv, bv, gamma)
    res = run_on_device(in_maps)
    return _unpack_out(res)
